# revision 40
# baseline (speedup 1.0000x reference)
"""Trainium2 Bass kernel for nn_MultiHeadEntityOPTAttention.

Multi-head attention with sparsemax over scores + entity-select combine.
Data-parallel over batch: 32 batches -> 8 NeuronCores x 4 batches, no
collectives.

v3: matmuls run in float32r (full-rate fp32); the sparsemax core runs in
fp16 where the DVE 4x perf mode applies.  Scores are copied PSUM->SBUF
fp16 by one tensor_tensor_reduce per (row-tile, head) that also adds the
-30000*mask offset and emits the row max (tau0 = rowmax - 1).  Each
Michelot/Newton step needs only two fast tensor_scalar reductions per
chunk: M = sum max(s,t) and c = #{s > t}; the support sum is recovered
as g = M - (256-c)*t and t' = (g-1)/c on tiny [128,16] tiles.  The
final relu pass (split DVE/ACT) writes fp16 attn and accumulates the
achieved row-sum, which normalizes the output and absorbs residual tau
error.  attn stays fp16 so the scale+transpose matmuls run at full PE
rate at free-size 128.  Front-batch work (projections/scores, PE-heavy)
is interleaved with back-batch sparsemax/output (DVE/ACT-heavy).

Self-contained: hardcodes all shapes; builds the Bass program once per
process and runs it SPMD on cores 0..7 via run_bass_kernel_spmd.
"""
import numpy as np
from contextlib import ExitStack

import concourse.bass as bass
import concourse.tile as tile
import concourse.mybir as mybir
from concourse import bacc
from concourse.masks import make_identity

F32 = mybir.dt.float32
F32R = mybir.dt.float32r
F16 = mybir.dt.float16
U8 = mybir.dt.uint8
AF = mybir.ActivationFunctionType
ALU = mybir.AluOpType
ts = bass.ts
ds = bass.ds

B, T, E, NH, NA = 32, 256, 256, 8, 64
NCORES = 8
BPC = B // NCORES          # batches per core
P = 128
QT = T // P                # 2 partition tiles along q
ET = E // P                # 2 tiles along e (contraction)
MASKVAL = -30000.0         # fp16-representable mask offset (post 1/16 scale)
N_MID = 3                  # Michelot/Newton (M,c) iterations
import os
USE_TTR = os.environ.get('KV3_TTR', '0') == '1'
USE_MTRICK = os.environ.get('KV3_MTRICK', '0') == '1'
USE_ACT_DIAG = os.environ.get('KV3_ACTDIAG', '0') == '1'
EXACT_FS = os.environ.get('KV3_EXACTFS', '0') == '1'
USE_VDMA = os.environ.get('KV3_VDMA', '1') == '1'
POOL_COUNTS = os.environ.get('KV3_POOLC', '1') == '1'
# mid-pass f-chunk -> engine (True = ACT Relu+accum, False = DVE STT)
MIDF_ACT = [i in (0, 1, 4, 7, 10, 13) for i in range(16)]  # 6 of 16, spread

# final-pass chunk -> engine: True = ACT (Relu+accum), False = DVE (STT)
FINAL_ACT = [False] * 16                        # final all DVE


def build_nc():
    nc = bacc.Bacc("TRN2", target_bir_lowering=False, debug=False,
                   num_devices=NCORES)
    x_d = nc.dram_tensor("x", [BPC, T, E], F32, kind="ExternalInput").ap()
    mask_d = nc.dram_tensor("mask", [BPC, T, T], U8, kind="ExternalInput").ap()
    wq_d = nc.dram_tensor("w_q", [E, NH * E], F32, kind="ExternalInput").ap()
    wk_d = nc.dram_tensor("w_k", [E, NH * E], F32, kind="ExternalInput").ap()
    wv_d = nc.dram_tensor("w_v", [E, NH * E], F32, kind="ExternalInput").ap()
    fsw_d = nc.dram_tensor("fc_select_w", [E, NH], F32, kind="ExternalInput").ap()
    fsb_d = nc.dram_tensor("fc_select_b", [1, NH], F32, kind="ExternalInput").ap()
    out_d = nc.dram_tensor("out", [BPC, T, E], F32, kind="ExternalOutput").ap()

    with tile.TileContext(nc) as tc, ExitStack() as ctx:
        const_pool = ctx.enter_context(tc.tile_pool(name="const", bufs=1))
        w_pool = ctx.enter_context(tc.tile_pool(name="weights", bufs=1))
        x_pool = ctx.enter_context(tc.tile_pool(name="x", bufs=2))
        mask_pool = ctx.enter_context(tc.tile_pool(name="mask", bufs=2))
        qk_pool = ctx.enter_context(tc.tile_pool(name="qk", bufs=2))
        v_pool = ctx.enter_context(tc.tile_pool(name="v", bufs=3))
        s16_pool = ctx.enter_context(tc.tile_pool(name="s16", bufs=2))
        attn_pool = ctx.enter_context(tc.tile_pool(name="attn", bufs=2))
        attnT_pool = ctx.enter_context(tc.tile_pool(name="attnT", bufs=4))
        uscr_pool = ctx.enter_context(tc.tile_pool(name="uscr", bufs=8))
        stats_pool = ctx.enter_context(tc.tile_pool(name="stats", bufs=3))
        sel_pool = ctx.enter_context(tc.tile_pool(name="sel", bufs=3))
        outf_pool = ctx.enter_context(tc.tile_pool(name="outf", bufs=2))

        mm_ps = ctx.enter_context(tc.tile_pool(name="mmps", bufs=4, space="PSUM"))
        sc_ps = ctx.enter_context(tc.tile_pool(name="scps", bufs=2, space="PSUM"))
        h0_ps = ctx.enter_context(tc.tile_pool(name="h0ps", bufs=1, space="PSUM"))
        mean_ps = ctx.enter_context(tc.tile_pool(name="meanps", bufs=1, space="PSUM"))

        # ---- constants / weights ----------------------------------------
        identf = const_pool.tile([P, P], F32)
        make_identity(nc, identf[:])
        ident16 = const_pool.tile([P, P], F16)
        make_identity(nc, ident16[:])
        zero16 = const_pool.tile([P, T], F16)
        nc.vector.memset(zero16[:], 0.0)
        ones_row = const_pool.tile([1, NA], F32)
        nc.vector.memset(ones_row[:], 1.0)
        invnh_ph = const_pool.tile([P, NH], F32)
        nc.vector.memset(invnh_ph[:], 1.0 / NH)

        wq = w_pool.tile([P, ET, NH * E], F32R)
        wk = w_pool.tile([P, ET, NH * E], F32R)
        wv = w_pool.tile([P, ET, NH * E], F32R)
        for h in range(NH):
            for w_sb, w_dram in ((wq, wq_d), (wk, wk_d), (wv, wv_d)):
                nc.gpsimd.dma_start(
                    w_sb[:, :, ds(h * E, E)],
                    w_dram[:, ds(h * E, E)].rearrange("(i p) n -> p i n", p=P))
        fsw = const_pool.tile([P, ET, NH], F32)
        nc.sync.dma_start(fsw[:], fsw_d.rearrange("(i p) n -> p i n", p=P))
        fsb = const_pool.tile([1, NH], F32)
        nc.sync.dma_start(fsb[:], fsb_d)

        def prep(b):
            """loads + mask prep + xT + select softmax inputs."""
            S = {}
            x32 = x_pool.tile([P, QT, E], F32, tag="x32")
            nc.sync.dma_start(x32[:], x_d[b].rearrange("(i p) e -> p i e", p=P))
            mask_u8 = mask_pool.tile([P, QT, T], U8, tag="m8")
            nc.sync.dma_start(mask_u8[:], mask_d[b].rearrange("(i p) k -> p i k", p=P))

            maskneg = mask_pool.tile([P, QT, T], F16, tag="mneg")
            nc.scalar.activation(maskneg[:], mask_u8[:], AF.Copy,
                                 bias=0.0, scale=MASKVAL)
            S['maskneg'] = maskneg

            xT = x_pool.tile([P, ET, T], F32R, tag="xT")
            xtp = mm_ps.tile([P, 2 * T], F32, tag="mm")
            for i in range(QT):
                for j in range(ET):
                    nc.tensor.transpose(xtp[:, ds(j * T + i * P, P)],
                                        x32[:, i, ts(j, P)], identf[:])
            nc.scalar.activation(xT[:], xtp[:].rearrange("p (i t) -> p i t", i=ET),
                                 AF.Copy, bias=0.0, scale=1.0)

            notmask = sel_pool.tile([NA, T], F32, tag="nm")
            nc.vector.tensor_scalar(out=notmask[:], in0=mask_u8[0:NA, 0, :],
                                    scalar1=-1.0, scalar2=1.0,
                                    op0=ALU.mult, op1=ALU.add)
            notmaskT = sel_pool.tile([P, QT, NA], F32, tag="nmT")
            nmp = mm_ps.tile([P, QT, NA], F32, tag="mm")
            for i in range(QT):
                nc.tensor.transpose(nmp[:, i, :], notmask[:, ts(i, P)],
                                    identf[0:NA, 0:NA])
            nc.vector.tensor_copy(notmaskT[:], nmp[:])

            xat = sel_pool.tile([P, ET, NA], F32, tag="xat")
            xatp = mm_ps.tile([P, ET, NA], F32, tag="mm")
            for j in range(ET):
                for i in range(QT):
                    nc.tensor.matmul(xatp[:, j, :], x32[:, i, ts(j, P)],
                                     notmaskT[:, i, :],
                                     start=(i == 0), stop=(i == QT - 1))
            nc.vector.tensor_copy(xat[:], xatp[:])

            logits = mm_ps.tile([NA, NH], F32, tag="mm")
            for j in range(ET):
                nc.tensor.matmul(logits[:], xat[:, j, :], fsw[:, j, :],
                                 start=(j == 0), stop=False)
            nc.tensor.matmul(logits[:], ones_row[:], fsb[:],
                             start=False, stop=True)
            selmx = sel_pool.tile([NA, 1], F32, tag="selmx")
            nc.vector.tensor_reduce(selmx[:], logits[:],
                                    axis=mybir.AxisListType.X, op=ALU.max,
                                    negate=True)
            sel_exp = sel_pool.tile([NA, NH], F32, tag="selexp")
            selsum = sel_pool.tile([NA, 1], F32, tag="selsum")
            nc.scalar.activation(sel_exp[:], logits[:], AF.Exp,
                                 bias=selmx[:], scale=1.0, accum_out=selsum[:])
            selrec = sel_pool.tile([NA, 1], F32, tag="selrec")
            nc.vector.reciprocal(selrec[:], selsum[:])
            sel = sel_pool.tile([NA, NH], F32, tag="sel")
            nc.vector.tensor_scalar_mul(sel[:], sel_exp[:], selrec[:])
            S['sel'] = sel
            S['xT'] = xT
            return S

        def alloc_qkv(S):
            S['q'] = qk_pool.tile([P, NH, ET, T], F16, tag="qk", name="q_all")
            S['k'] = qk_pool.tile([P, NH, ET, T], F16, tag="qk", name="k_all")
            S['v'] = v_pool.tile([P, NH, QT, E], F16, tag="v", name="v_all")

        def qkv_head(S, h):
            xT = S['xT']
            qp = mm_ps.tile([P, ET, T], F32, tag="mm")
            for j in range(ET):
                for i in range(ET):
                    nc.tensor.matmul(qp[:, j, :], wq[:, i, ds(h * E + j * P, P)],
                                     xT[:, i, :], start=(i == 0),
                                     stop=(i == ET - 1))
            nc.scalar.activation(S['q'][:, h, :, :], qp[:], AF.Copy,
                                 bias=0.0, scale=1.0 / 16.0)
            kp = mm_ps.tile([P, ET, T], F32, tag="mm")
            for j in range(ET):
                for i in range(ET):
                    nc.tensor.matmul(kp[:, j, :], wk[:, i, ds(h * E + j * P, P)],
                                     xT[:, i, :], start=(i == 0),
                                     stop=(i == ET - 1))
            nc.scalar.activation(S['k'][:, h, :, :], kp[:], AF.Copy,
                                 bias=0.0, scale=1.0)
            vp = mm_ps.tile([P, QT, E], F32, tag="mm")
            for i in range(QT):
                for j in range(ET):
                    nc.tensor.matmul(vp[:, i, :], xT[:, j, ts(i, P)],
                                     wv[:, j, ds(h * E, E)],
                                     start=(j == 0), stop=(j == ET - 1))
            nc.scalar.activation(S['v'][:, h, :, :], vp[:], AF.Copy,
                                 bias=0.0, scale=1.0)

        def alloc_sparse(S):
            S['s16'] = s16_pool.tile([P, QT, NH, T], F16, tag="s16", name="s16")
            S['nmx'] = stats_pool.tile([P, QT, NH], F32, tag="nmx", name="nmx")
            S['ptau'] = stats_pool.tile([P, QT, NH], F32, tag="ptau", name="ptau")
            S['Mac'] = stats_pool.tile([P, QT, NH], F32, tag="Mac", name="Mac")
            S['cst'] = stats_pool.tile([P, QT, NH], F32, tag="cst", name="cst")
            S['fst'] = stats_pool.tile([P, QT, NH], F32, tag="fst", name="fst")

        def scores_chunk(S, qt, h2):
            """scores for heads (2*h2, 2*h2+1) at row tile qt; fused
            mask-add / fp16-store / rowmax via tensor_tensor_reduce."""
            sc = sc_ps.tile([P, 2, T], F32, tag="sc")
            for hh in range(2):
                h = h2 * 2 + hh
                if not USE_TTR:
                    nc.tensor.matmul(sc[:, hh, :], ident16[:],
                                     S['maskneg'][:, qt, :],
                                     start=True, stop=False)
                for i in range(ET):
                    nc.tensor.matmul(sc[:, hh, :], S['q'][:, h, i, ts(qt, P)],
                                     S['k'][:, h, i, :],
                                     start=(USE_TTR and i == 0),
                                     stop=(i == ET - 1))
            if USE_TTR:
                for hh in range(2):
                    h = h2 * 2 + hh
                    nc.vector.tensor_tensor_reduce(
                        out=S['s16'][:, qt, h, :], in0=sc[:, hh, :],
                        in1=S['maskneg'][:, qt, :], scale=1.0, scalar=-1.0e30,
                        op0=ALU.add, op1=ALU.max,
                        accum_out=S['nmx'][:, qt, h:h + 1])
            else:
                nc.scalar.activation(
                    S['s16'][:, qt, ds(h2 * 2, 2), :], sc[:],
                    AF.Copy, bias=0.0, scale=1.0)
                nc.vector.tensor_reduce(
                    S['nmx'][:, qt, ds(h2 * 2, 2)], sc[:],
                    axis=mybir.AxisListType.X, op=ALU.max)

        def sparse_init(S):
            # tau0 = rowmax - 1
            nc.vector.tensor_scalar(out=S['ptau'][:], in0=S['nmx'][:],
                                    scalar1=-1.0, scalar2=None, op0=ALU.add)
            ntau = stats_pool.tile([P, QT, NH], F32, tag="ntau", name="ntau")
            nc.vector.tensor_scalar(out=ntau[:], in0=S['nmx'][:],
                                    scalar1=-1.0, scalar2=1.0,
                                    op0=ALU.mult, op1=ALU.add)
            S['ntau'] = ntau
            # live-row flag from head 0's rowmax (mask shared across heads)
            notrow = stats_pool.tile([P, QT], F32, tag="notrow", name="notrow")
            nc.vector.tensor_scalar(out=notrow[:], in0=S['nmx'][:, :, 0],
                                    scalar1=-1.0e4, scalar2=None, op0=ALU.is_gt)
            S['notrow'] = notrow

        def mc_chunk(S, qt, h, idx):
            """f = sum relu(s-t) (STT on DVE / Relu on ACT) and c = #{s>t}."""
            mscr = uscr_pool.tile([P, T], F16, tag="uscr", name="mscr")
            if USE_MTRICK:
                nc.vector.tensor_scalar(
                    out=mscr[:], in0=S['s16'][:, qt, h, :],
                    scalar1=S['ptau'][:, qt, h:h + 1], scalar2=None,
                    op0=ALU.max, op1=ALU.add,
                    accum_out=S['Mac'][:, qt, h:h + 1])
            elif MIDF_ACT[idx]:
                nc.scalar.activation(mscr[:], S['s16'][:, qt, h, :], AF.Relu,
                                     bias=S['ntau'][:, qt, h:h + 1], scale=1.0,
                                     accum_out=S['Mac'][:, qt, h:h + 1])
            else:
                nc.vector.scalar_tensor_tensor(
                    out=mscr[:], in0=S['s16'][:, qt, h, :],
                    scalar=S['ntau'][:, qt, h:h + 1], in1=zero16[:],
                    op0=ALU.add, op1=ALU.max,
                    accum_out=S['Mac'][:, qt, h:h + 1])
            cscr = uscr_pool.tile([P, T], F16, tag="uscr", name="cscr")
            ceng = nc.gpsimd if POOL_COUNTS else nc.vector
            ceng.tensor_scalar(
                out=cscr[:], in0=S['s16'][:, qt, h, :],
                scalar1=S['ptau'][:, qt, h:h + 1], scalar2=None,
                op0=ALU.is_gt, op1=ALU.add,
                accum_out=S['cst'][:, qt, h:h + 1])

        def newton_update(S):
            # f-semantics: t' = t + (f-1)/c
            recipc = stats_pool.tile([P, QT, NH], F32, tag="recipc")
            delta = stats_pool.tile([P, QT, NH], F32, tag="delta")
            nc.vector.reciprocal(recipc[:], S['cst'][:])
            nc.vector.scalar_tensor_tensor(
                out=delta[:], in0=S['Mac'][:], scalar=-1.0,
                in1=recipc[:], op0=ALU.add, op1=ALU.mult)
            nc.vector.tensor_tensor(out=S['ptau'][:], in0=S['ptau'][:],
                                    in1=delta[:], op=ALU.add)
            nc.vector.tensor_scalar_mul(S['ntau'][:], S['ptau'][:], -1.0)

        def final_chunk(S, qt, h, on_act):
            """attn = relu(s - t) (fp16); Michelot's last update makes the
            support sum 1 exactly for converged rows, so no fs accumulate."""
            out = S['attn'][:, qt, h, :]
            if EXACT_FS:
                if on_act:
                    nc.scalar.activation(out, S['s16'][:, qt, h, :], AF.Relu,
                                         bias=S['ntau'][:, qt, h:h + 1],
                                         scale=1.0,
                                         accum_out=S['fst'][:, qt, h:h + 1])
                else:
                    nc.vector.scalar_tensor_tensor(
                        out=out, in0=S['s16'][:, qt, h, :],
                        scalar=S['ntau'][:, qt, h:h + 1], in1=zero16[:],
                        op0=ALU.add, op1=ALU.max,
                        accum_out=S['fst'][:, qt, h:h + 1])
            else:
                nc.vector.tensor_scalar(
                    out=out, in0=S['s16'][:, qt, h, :],
                    scalar1=S['ntau'][:, qt, h:h + 1], scalar2=0.0,
                    op0=ALU.add, op1=ALU.max)

        def sparse_piece(S, piece):
            """8 pieces: (f,c)x3 iterations (2 pieces each) + final (2).
            Piece = (iteration j, qt half); the tau update for half q
            runs as soon as that half's chunks land."""
            j = piece // 2
            half = piece % 2
            if j < N_MID:
                if j > 0 and half == 0:
                    newton_update(S)
                for t in range(8):
                    idx = half * 8 + t
                    mc_chunk(S, idx // NH, idx % NH, idx)
            else:
                if half == 0:
                    newton_update(S)
                    S['attn'] = attn_pool.tile([P, QT, NH, T], F16, tag="attn",
                                               name="attn")
                for t in range(8):
                    idx = half * 8 + t
                    final_chunk(S, idx // NH, idx % NH, FINAL_ACT[idx])

        def normalize(S):
            dall = stats_pool.tile([P, QT, NH], F32, tag="dall")
            if EXACT_FS:
                recipf = stats_pool.tile([P, QT, NH], F32, tag="recipf")
                nc.vector.reciprocal(recipf[:], S['fst'][:])
                for qt in range(QT):
                    nc.vector.tensor_scalar_mul(recipf[:, qt, :],
                                                recipf[:, qt, :],
                                                S['notrow'][:, qt:qt + 1])
                nc.vector.tensor_tensor(out=dall[0:NA, 0, :],
                                        in0=recipf[0:NA, 0, :],
                                        in1=S['sel'][:], op=ALU.mult)
                nc.vector.tensor_scalar_mul(dall[ds(NA, NA), 0, :],
                                            recipf[ds(NA, NA), 0, :], 1.0 / NH)
                nc.vector.tensor_scalar_mul(dall[:, 1, :], recipf[:, 1, :],
                                            1.0 / NH)
            else:
                # fs == 1 by construction: scales are notrow * (sel | 1/NH)
                for qt in range(QT):
                    nc.vector.tensor_scalar_mul(dall[:, qt, :], invnh_ph[:],
                                                S['notrow'][:, qt:qt + 1])
                nc.vector.tensor_scalar_mul(dall[0:NA, 0, :], S['sel'][:],
                                            S['notrow'][0:NA, 0:1])
            S['dall'] = dall
            S['outf'] = outf_pool.tile([P, QT, E], F32, tag="outf", name="outf")
            S['hm'] = h0_ps.tile([P, E], F32, tag="h0", name="hm")
            S['mean1'] = mean_ps.tile([P, E], F32, tag="mean", name="mean1")

        def out_head(S, h, last=False):
            attn, v_all = S['attn'], S['v']
            attnT = attnT_pool.tile([P, QT, T], F16, tag="attnT")
            atp = mm_ps.tile([P, QT, T], F32, tag="mm")
            diag = uscr_pool.tile([P, QT, P], F16, tag="diag", name="diag")
            for qt in range(QT):
                if last and qt == 1:
                    nc.vector.tensor_scalar_mul(diag[:, qt, :], ident16[:],
                                                S['dall'][:, qt, h:h + 1])
                else:
                    nc.scalar.activation(diag[:, qt, :], ident16[:], AF.Copy,
                                         bias=0.0,
                                         scale=S['dall'][:, qt, h:h + 1])
            for ki in range(QT):
                for qt in range(QT):
                    nc.tensor.matmul(atp[:, ki, ts(qt, P)],
                                     attn[:, qt, h, ts(ki, P)],
                                     diag[:, qt, :], start=True, stop=True)
            if last and h % 2 == 1:
                nc.vector.tensor_copy(attnT[:], atp[:])
            else:
                nc.scalar.activation(attnT[:], atp[:], AF.Copy,
                                     bias=0.0, scale=1.0)
            for ki in range(QT):
                nc.tensor.matmul(S['hm'][:], attnT[:, ki, 0:P],
                                 v_all[:, h, ki, :],
                                 start=(h == 0 and ki == 0),
                                 stop=(h == NH - 1 and ki == QT - 1))
            for ki in range(QT):
                nc.tensor.matmul(S['mean1'][:], attnT[:, ki, ts(1, P)],
                                 v_all[:, h, ki, :],
                                 start=(h == 0 and ki == 0),
                                 stop=(h == NH - 1 and ki == QT - 1))

        def finish(b, S):
            nc.vector.tensor_copy(S['outf'][0:NA, 0, :], S['hm'][0:NA, :])
            nc.scalar.activation(S['outf'][ds(NA, NA), 0, :],
                                 S['hm'][ds(NA, NA), :],
                                 AF.Copy, bias=0.0, scale=1.0)
            nc.scalar.activation(S['outf'][:, 1, :], S['mean1'][:],
                                 AF.Copy, bias=0.0, scale=1.0)
            nc.sync.dma_start(out_d[b].rearrange("(i p) e -> p i e", p=P),
                              S['outf'][:])

        # ---- 3-deep skewed pipeline -------------------------------------
        # step s: qkv+scores(s) | sparsemax(s-1) | output(s-2)
        st = [None] * BPC
        for s in range(BPC + 2):
            F = s < BPC                       # front batch exists
            M = 1 <= s <= BPC                 # mid (sparse) batch exists
            Bk = s >= 2                       # back (output) batch exists
            if F:
                st[s] = prep(s)
                alloc_qkv(st[s])
                alloc_sparse(st[s])
            if M:
                sparse_init(st[s - 1])
            if Bk:
                normalize(st[s - 2])
            for h in range(NH):
                if F:
                    qkv_head(st[s], h)
                    if h % 2 == 1:
                        scores_chunk(st[s], 0, h // 2)
                        scores_chunk(st[s], 1, h // 2)
                if M:
                    if s == BPC:
                        # no front work: compress sparse into slots 0..3 so
                        # the final batch's attn is ready before the drain
                        if h < 4:
                            sparse_piece(st[s - 1], 2 * h)
                            sparse_piece(st[s - 1], 2 * h + 1)
                    else:
                        sparse_piece(st[s - 1], h)
                if Bk:
                    out_head(st[s - 2], h, last=(s == BPC + 1))
            if Bk:
                finish(s - 2, st[s - 2])
                st[s - 2] = None
    nc.compile()
    return nc


_NC_CACHE = None


def _get_nc():
    global _NC_CACHE
    if _NC_CACHE is None:
        _NC_CACHE = build_nc()
    return _NC_CACHE


def make_in_maps(x, mask, w_q, w_k, w_v, fc_select_w, fc_select_b):
    mask_u8 = np.ascontiguousarray(mask).view(np.uint8)
    in_maps = []
    for c in range(NCORES):
        sl = slice(c * BPC, (c + 1) * BPC)
        in_maps.append({
            "x": np.ascontiguousarray(x[sl], dtype=np.float32),
            "mask": np.ascontiguousarray(mask_u8[sl]),
            "w_q": np.ascontiguousarray(w_q, dtype=np.float32),
            "w_k": np.ascontiguousarray(w_k, dtype=np.float32),
            "w_v": np.ascontiguousarray(w_v, dtype=np.float32),
            "fc_select_w": np.ascontiguousarray(fc_select_w, dtype=np.float32),
            "fc_select_b": np.ascontiguousarray(
                fc_select_b, dtype=np.float32).reshape(1, NH),
        })
    return in_maps


def kernel(x, h, mask, w_q, w_k, w_v, fc_select_w, fc_select_b, **kwargs):
    from concourse import bass_utils
    nc = _get_nc()
    in_maps = make_in_maps(x, mask, w_q, w_k, w_v, fc_select_w, fc_select_b)
    res = bass_utils.run_bass_kernel_spmd(nc, in_maps,
                                          core_ids=list(range(NCORES)))
    out = np.concatenate([res.results[c]["out"] for c in range(NCORES)], axis=0)
    return out.astype(np.float32)


# revision 41
# speedup vs baseline: 1.0153x; 1.0153x over previous
"""Trainium2 Bass kernel for nn_MultiHeadEntityOPTAttention.

Multi-head attention with sparsemax over scores + entity-select combine.
Data-parallel over batch: 32 batches -> 8 NeuronCores x 4 batches, no
collectives.

v3: matmuls run in float32r (full-rate fp32); the sparsemax core runs in
fp16 where the DVE 4x perf mode applies.  Scores are copied PSUM->SBUF
fp16 by one tensor_tensor_reduce per (row-tile, head) that also adds the
-30000*mask offset and emits the row max (tau0 = rowmax - 1).  Each
Michelot/Newton step needs only two fast tensor_scalar reductions per
chunk: M = sum max(s,t) and c = #{s > t}; the support sum is recovered
as g = M - (256-c)*t and t' = (g-1)/c on tiny [128,16] tiles.  The
final relu pass (split DVE/ACT) writes fp16 attn and accumulates the
achieved row-sum, which normalizes the output and absorbs residual tau
error.  attn stays fp16 so the scale+transpose matmuls run at full PE
rate at free-size 128.  Front-batch work (projections/scores, PE-heavy)
is interleaved with back-batch sparsemax/output (DVE/ACT-heavy).

Self-contained: hardcodes all shapes; builds the Bass program once per
process and runs it SPMD on cores 0..7 via run_bass_kernel_spmd.
"""
import numpy as np
from contextlib import ExitStack

import concourse.bass as bass
import concourse.tile as tile
import concourse.mybir as mybir
from concourse import bacc
from concourse.masks import make_identity

F32 = mybir.dt.float32
F32R = mybir.dt.float32r
F16 = mybir.dt.float16
U8 = mybir.dt.uint8
AF = mybir.ActivationFunctionType
ALU = mybir.AluOpType
ts = bass.ts
ds = bass.ds

B, T, E, NH, NA = 32, 256, 256, 8, 64
NCORES = 8
BPC = B // NCORES          # batches per core
P = 128
QT = T // P                # 2 partition tiles along q
ET = E // P                # 2 tiles along e (contraction)
MASKVAL = -30000.0         # fp16-representable mask offset (post 1/16 scale)
N_MID = 3                  # Michelot/Newton (M,c) iterations
import os
USE_TTR = os.environ.get('KV3_TTR', '0') == '1'
USE_MTRICK = os.environ.get('KV3_MTRICK', '0') == '1'
USE_ACT_DIAG = os.environ.get('KV3_ACTDIAG', '0') == '1'
EXACT_FS = os.environ.get('KV3_EXACTFS', '0') == '1'
USE_VDMA = os.environ.get('KV3_VDMA', '1') == '1'
POOL_COUNTS = os.environ.get('KV3_POOLC', '1') == '1'
# mid-pass f-chunk -> engine (True = ACT Relu+accum, False = DVE STT)
MIDF_ACT = [i in (0, 1, 4, 7, 10, 13) for i in range(16)]  # 6 of 16, spread

# final-pass chunk -> engine: True = ACT (Relu+accum), False = DVE (STT)
FINAL_ACT = [False] * 16                        # final all DVE


def build_nc():
    nc = bacc.Bacc("TRN2", target_bir_lowering=False, debug=False,
                   num_devices=NCORES)
    x_d = nc.dram_tensor("x", [BPC, T, E], F32, kind="ExternalInput").ap()
    mask_d = nc.dram_tensor("mask", [BPC, T, T], U8, kind="ExternalInput").ap()
    wq_d = nc.dram_tensor("w_q", [E, NH * E], F32, kind="ExternalInput").ap()
    wk_d = nc.dram_tensor("w_k", [E, NH * E], F32, kind="ExternalInput").ap()
    wv_d = nc.dram_tensor("w_v", [E, NH * E], F32, kind="ExternalInput").ap()
    fsw_d = nc.dram_tensor("fc_select_w", [E, NH], F32, kind="ExternalInput").ap()
    fsb_d = nc.dram_tensor("fc_select_b", [1, NH], F32, kind="ExternalInput").ap()
    out_d = nc.dram_tensor("out", [BPC, T, E], F32, kind="ExternalOutput").ap()

    with tile.TileContext(nc) as tc, ExitStack() as ctx:
        const_pool = ctx.enter_context(tc.tile_pool(name="const", bufs=1))
        w_pool = ctx.enter_context(tc.tile_pool(name="weights", bufs=1))
        x_pool = ctx.enter_context(tc.tile_pool(name="x", bufs=2))
        mask_pool = ctx.enter_context(tc.tile_pool(name="mask", bufs=2))
        qk_pool = ctx.enter_context(tc.tile_pool(name="qk", bufs=2))
        v_pool = ctx.enter_context(tc.tile_pool(name="v", bufs=3))
        s16_pool = ctx.enter_context(tc.tile_pool(name="s16", bufs=2))
        attn_pool = ctx.enter_context(tc.tile_pool(name="attn", bufs=2))
        attnT_pool = ctx.enter_context(tc.tile_pool(name="attnT", bufs=4))
        uscr_pool = ctx.enter_context(tc.tile_pool(name="uscr", bufs=8))
        stats_pool = ctx.enter_context(tc.tile_pool(name="stats", bufs=3))
        sel_pool = ctx.enter_context(tc.tile_pool(name="sel", bufs=3))
        outf_pool = ctx.enter_context(tc.tile_pool(name="outf", bufs=2))

        mm_ps = ctx.enter_context(tc.tile_pool(name="mmps", bufs=4, space="PSUM"))
        sc_ps = ctx.enter_context(tc.tile_pool(name="scps", bufs=2, space="PSUM"))
        h0_ps = ctx.enter_context(tc.tile_pool(name="h0ps", bufs=1, space="PSUM"))
        mean_ps = ctx.enter_context(tc.tile_pool(name="meanps", bufs=1, space="PSUM"))

        # ---- constants / weights ----------------------------------------
        identf = const_pool.tile([P, P], F32)
        make_identity(nc, identf[:])
        ident16 = const_pool.tile([P, P], F16)
        make_identity(nc, ident16[:])
        zero16 = const_pool.tile([P, T], F16)
        nc.vector.memset(zero16[:], 0.0)
        ones_row = const_pool.tile([1, NA], F32)
        nc.vector.memset(ones_row[:], 1.0)
        invnh_ph = const_pool.tile([P, NH], F32)
        nc.vector.memset(invnh_ph[:], 1.0 / NH)

        wq = w_pool.tile([P, ET, NH * E], F32R)
        wk = w_pool.tile([P, ET, NH * E], F32R)
        wv = w_pool.tile([P, ET, NH * E], F32R)
        for h in range(NH):
            for w_sb, w_dram in ((wq, wq_d), (wk, wk_d), (wv, wv_d)):
                nc.gpsimd.dma_start(
                    w_sb[:, :, ds(h * E, E)],
                    w_dram[:, ds(h * E, E)].rearrange("(i p) n -> p i n", p=P))
        fsw = const_pool.tile([P, ET, NH], F32)
        nc.sync.dma_start(fsw[:], fsw_d.rearrange("(i p) n -> p i n", p=P))
        fsb = const_pool.tile([1, NH], F32)
        nc.sync.dma_start(fsb[:], fsb_d)

        def prep(b):
            """loads + mask prep + xT + select softmax inputs."""
            S = {}
            x32 = x_pool.tile([P, QT, E], F32, tag="x32")
            nc.sync.dma_start(x32[:], x_d[b].rearrange("(i p) e -> p i e", p=P))
            mask_u8 = mask_pool.tile([P, QT, T], U8, tag="m8")
            nc.sync.dma_start(mask_u8[:], mask_d[b].rearrange("(i p) k -> p i k", p=P))

            maskneg = mask_pool.tile([P, QT, T], F16, tag="mneg")
            nc.scalar.activation(maskneg[:], mask_u8[:], AF.Copy,
                                 bias=0.0, scale=MASKVAL)
            S['maskneg'] = maskneg

            xT = x_pool.tile([P, ET, T], F32R, tag="xT")
            xtp = mm_ps.tile([P, 2 * T], F32, tag="mm")
            for i in range(QT):
                for j in range(ET):
                    nc.tensor.transpose(xtp[:, ds(j * T + i * P, P)],
                                        x32[:, i, ts(j, P)], identf[:])
            nc.scalar.activation(xT[:], xtp[:].rearrange("p (i t) -> p i t", i=ET),
                                 AF.Copy, bias=0.0, scale=1.0)

            notmask = sel_pool.tile([NA, T], F32, tag="nm")
            nc.vector.tensor_scalar(out=notmask[:], in0=mask_u8[0:NA, 0, :],
                                    scalar1=-1.0, scalar2=1.0,
                                    op0=ALU.mult, op1=ALU.add)
            notmaskT = sel_pool.tile([P, QT, NA], F32, tag="nmT")
            nmp = mm_ps.tile([P, QT, NA], F32, tag="mm")
            for i in range(QT):
                nc.tensor.transpose(nmp[:, i, :], notmask[:, ts(i, P)],
                                    identf[0:NA, 0:NA])
            nc.vector.tensor_copy(notmaskT[:], nmp[:])

            xat = sel_pool.tile([P, ET, NA], F32, tag="xat")
            xatp = mm_ps.tile([P, ET, NA], F32, tag="mm")
            for j in range(ET):
                for i in range(QT):
                    nc.tensor.matmul(xatp[:, j, :], x32[:, i, ts(j, P)],
                                     notmaskT[:, i, :],
                                     start=(i == 0), stop=(i == QT - 1))
            nc.vector.tensor_copy(xat[:], xatp[:])

            logits = mm_ps.tile([NA, NH], F32, tag="mm")
            for j in range(ET):
                nc.tensor.matmul(logits[:], xat[:, j, :], fsw[:, j, :],
                                 start=(j == 0), stop=False)
            nc.tensor.matmul(logits[:], ones_row[:], fsb[:],
                             start=False, stop=True)
            selmx = sel_pool.tile([NA, 1], F32, tag="selmx")
            nc.vector.tensor_reduce(selmx[:], logits[:],
                                    axis=mybir.AxisListType.X, op=ALU.max,
                                    negate=True)
            sel_exp = sel_pool.tile([NA, NH], F32, tag="selexp")
            selsum = sel_pool.tile([NA, 1], F32, tag="selsum")
            nc.scalar.activation(sel_exp[:], logits[:], AF.Exp,
                                 bias=selmx[:], scale=1.0, accum_out=selsum[:])
            selrec = sel_pool.tile([NA, 1], F32, tag="selrec")
            nc.vector.reciprocal(selrec[:], selsum[:])
            sel = sel_pool.tile([NA, NH], F32, tag="sel")
            nc.vector.tensor_scalar_mul(sel[:], sel_exp[:], selrec[:])
            S['sel'] = sel
            S['xT'] = xT
            return S

        def alloc_qkv(S):
            S['q'] = qk_pool.tile([P, NH, ET, T], F16, tag="qk", name="q_all")
            S['k'] = qk_pool.tile([P, NH, ET, T], F16, tag="qk", name="k_all")
            S['v'] = v_pool.tile([P, NH, QT, E], F16, tag="v", name="v_all")

        def qkv_head(S, h):
            xT = S['xT']
            qp = mm_ps.tile([P, ET, T], F32, tag="mm")
            for j in range(ET):
                for i in range(ET):
                    nc.tensor.matmul(qp[:, j, :], wq[:, i, ds(h * E + j * P, P)],
                                     xT[:, i, :], start=(i == 0),
                                     stop=(i == ET - 1))
            nc.scalar.activation(S['q'][:, h, :, :], qp[:], AF.Copy,
                                 bias=0.0, scale=1.0 / 16.0)
            kp = mm_ps.tile([P, ET, T], F32, tag="mm")
            for j in range(ET):
                for i in range(ET):
                    nc.tensor.matmul(kp[:, j, :], wk[:, i, ds(h * E + j * P, P)],
                                     xT[:, i, :], start=(i == 0),
                                     stop=(i == ET - 1))
            nc.scalar.activation(S['k'][:, h, :, :], kp[:], AF.Copy,
                                 bias=0.0, scale=1.0)
            vp = mm_ps.tile([P, QT, E], F32, tag="mm")
            for i in range(QT):
                for j in range(ET):
                    nc.tensor.matmul(vp[:, i, :], xT[:, j, ts(i, P)],
                                     wv[:, j, ds(h * E, E)],
                                     start=(j == 0), stop=(j == ET - 1))
            nc.scalar.activation(S['v'][:, h, :, :], vp[:], AF.Copy,
                                 bias=0.0, scale=1.0)

        def alloc_sparse(S):
            S['s16'] = s16_pool.tile([P, QT, NH, T], F16, tag="s16", name="s16")
            S['nmx'] = stats_pool.tile([P, QT, NH], F32, tag="nmx", name="nmx")
            S['ptau'] = stats_pool.tile([P, QT, NH], F32, tag="ptau", name="ptau")
            S['Mac'] = stats_pool.tile([P, QT, NH], F32, tag="Mac", name="Mac")
            S['cst'] = stats_pool.tile([P, QT, NH], F32, tag="cst", name="cst")
            S['fst'] = stats_pool.tile([P, QT, NH], F32, tag="fst", name="fst")

        def scores_chunk(S, qt, h2):
            """scores for heads (2*h2, 2*h2+1) at row tile qt; fused
            mask-add / fp16-store / rowmax via tensor_tensor_reduce."""
            sc = sc_ps.tile([P, 2, T], F32, tag="sc")
            for hh in range(2):
                h = h2 * 2 + hh
                if not USE_TTR:
                    nc.tensor.matmul(sc[:, hh, :], ident16[:],
                                     S['maskneg'][:, qt, :],
                                     start=True, stop=False)
                for i in range(ET):
                    nc.tensor.matmul(sc[:, hh, :], S['q'][:, h, i, ts(qt, P)],
                                     S['k'][:, h, i, :],
                                     start=(USE_TTR and i == 0),
                                     stop=(i == ET - 1))
            if USE_TTR:
                for hh in range(2):
                    h = h2 * 2 + hh
                    nc.vector.tensor_tensor_reduce(
                        out=S['s16'][:, qt, h, :], in0=sc[:, hh, :],
                        in1=S['maskneg'][:, qt, :], scale=1.0, scalar=-1.0e30,
                        op0=ALU.add, op1=ALU.max,
                        accum_out=S['nmx'][:, qt, h:h + 1])
            else:
                nc.scalar.activation(
                    S['s16'][:, qt, ds(h2 * 2, 2), :], sc[:],
                    AF.Copy, bias=0.0, scale=1.0)
                nc.vector.tensor_reduce(
                    S['nmx'][:, qt, ds(h2 * 2, 2)], sc[:],
                    axis=mybir.AxisListType.X, op=ALU.max)

        def sparse_init(S):
            # tau0 = rowmax - 1
            nc.vector.tensor_scalar(out=S['ptau'][:], in0=S['nmx'][:],
                                    scalar1=-1.0, scalar2=None, op0=ALU.add)
            ntau = stats_pool.tile([P, QT, NH], F32, tag="ntau", name="ntau")
            nc.vector.tensor_scalar(out=ntau[:], in0=S['nmx'][:],
                                    scalar1=-1.0, scalar2=1.0,
                                    op0=ALU.mult, op1=ALU.add)
            S['ntau'] = ntau
            # live-row flag from head 0's rowmax (mask shared across heads)
            notrow = stats_pool.tile([P, QT], F32, tag="notrow", name="notrow")
            nc.vector.tensor_scalar(out=notrow[:], in0=S['nmx'][:, :, 0],
                                    scalar1=-1.0e4, scalar2=None, op0=ALU.is_gt)
            S['notrow'] = notrow

        def mc_chunk(S, qt, h, idx):
            """f = sum relu(s-t) (STT on DVE / Relu on ACT) and c = #{s>t}."""
            mscr = uscr_pool.tile([P, T], F16, tag="uscr", name="mscr")
            if USE_MTRICK:
                nc.vector.tensor_scalar(
                    out=mscr[:], in0=S['s16'][:, qt, h, :],
                    scalar1=S['ptau'][:, qt, h:h + 1], scalar2=None,
                    op0=ALU.max, op1=ALU.add,
                    accum_out=S['Mac'][:, qt, h:h + 1])
            elif MIDF_ACT[idx]:
                nc.scalar.activation(mscr[:], S['s16'][:, qt, h, :], AF.Relu,
                                     bias=S['ntau'][:, qt, h:h + 1], scale=1.0,
                                     accum_out=S['Mac'][:, qt, h:h + 1])
            else:
                nc.vector.scalar_tensor_tensor(
                    out=mscr[:], in0=S['s16'][:, qt, h, :],
                    scalar=S['ntau'][:, qt, h:h + 1], in1=zero16[:],
                    op0=ALU.add, op1=ALU.max,
                    accum_out=S['Mac'][:, qt, h:h + 1])
            cscr = uscr_pool.tile([P, T], F16, tag="uscr", name="cscr")
            ceng = nc.gpsimd if POOL_COUNTS else nc.vector
            ceng.tensor_scalar(
                out=cscr[:], in0=S['s16'][:, qt, h, :],
                scalar1=S['ptau'][:, qt, h:h + 1], scalar2=None,
                op0=ALU.is_gt, op1=ALU.add,
                accum_out=S['cst'][:, qt, h:h + 1])

        def newton_update(S):
            # f-semantics: t' = t + (f-1)/c
            recipc = stats_pool.tile([P, QT, NH], F32, tag="recipc")
            delta = stats_pool.tile([P, QT, NH], F32, tag="delta")
            nc.vector.reciprocal(recipc[:], S['cst'][:])
            nc.vector.scalar_tensor_tensor(
                out=delta[:], in0=S['Mac'][:], scalar=-1.0,
                in1=recipc[:], op0=ALU.add, op1=ALU.mult)
            nc.vector.tensor_tensor(out=S['ptau'][:], in0=S['ptau'][:],
                                    in1=delta[:], op=ALU.add)
            nc.vector.tensor_scalar_mul(S['ntau'][:], S['ptau'][:], -1.0)

        def final_chunk(S, qt, h, on_act):
            """attn = relu(s - t) (fp16); Michelot's last update makes the
            support sum 1 exactly for converged rows, so no fs accumulate."""
            out = S['attn'][:, qt, h, :]
            if EXACT_FS:
                if on_act:
                    nc.scalar.activation(out, S['s16'][:, qt, h, :], AF.Relu,
                                         bias=S['ntau'][:, qt, h:h + 1],
                                         scale=1.0,
                                         accum_out=S['fst'][:, qt, h:h + 1])
                else:
                    nc.vector.scalar_tensor_tensor(
                        out=out, in0=S['s16'][:, qt, h, :],
                        scalar=S['ntau'][:, qt, h:h + 1], in1=zero16[:],
                        op0=ALU.add, op1=ALU.max,
                        accum_out=S['fst'][:, qt, h:h + 1])
            else:
                nc.vector.tensor_scalar(
                    out=out, in0=S['s16'][:, qt, h, :],
                    scalar1=S['ntau'][:, qt, h:h + 1], scalar2=0.0,
                    op0=ALU.add, op1=ALU.max)

        def sparse_piece(S, piece):
            """8 pieces: (f,c)x3 iterations (2 pieces each) + final (2).
            Piece = (iteration j, qt half); the tau update for half q
            runs as soon as that half's chunks land."""
            j = piece // 2
            half = piece % 2
            if j < N_MID:
                if j > 0 and half == 0:
                    newton_update(S)
                for t in range(8):
                    idx = half * 8 + t
                    mc_chunk(S, idx // NH, idx % NH, idx)
            else:
                if half == 0:
                    newton_update(S)
                    S['attn'] = attn_pool.tile([P, QT, NH, T], F16, tag="attn",
                                               name="attn")
                for t in range(8):
                    idx = half * 8 + t
                    final_chunk(S, idx // NH, idx % NH, FINAL_ACT[idx])

        def normalize(S):
            dall = stats_pool.tile([P, QT, NH], F32, tag="dall")
            if EXACT_FS:
                recipf = stats_pool.tile([P, QT, NH], F32, tag="recipf")
                nc.vector.reciprocal(recipf[:], S['fst'][:])
                for qt in range(QT):
                    nc.vector.tensor_scalar_mul(recipf[:, qt, :],
                                                recipf[:, qt, :],
                                                S['notrow'][:, qt:qt + 1])
                nc.vector.tensor_tensor(out=dall[0:NA, 0, :],
                                        in0=recipf[0:NA, 0, :],
                                        in1=S['sel'][:], op=ALU.mult)
                nc.vector.tensor_scalar_mul(dall[ds(NA, NA), 0, :],
                                            recipf[ds(NA, NA), 0, :], 1.0 / NH)
                nc.vector.tensor_scalar_mul(dall[:, 1, :], recipf[:, 1, :],
                                            1.0 / NH)
            else:
                # fs == 1 by construction: scales are notrow * (sel | 1/NH)
                for qt in range(QT):
                    nc.vector.tensor_scalar_mul(dall[:, qt, :], invnh_ph[:],
                                                S['notrow'][:, qt:qt + 1])
                nc.vector.tensor_scalar_mul(dall[0:NA, 0, :], S['sel'][:],
                                            S['notrow'][0:NA, 0:1])
            S['dall'] = dall
            S['outf'] = outf_pool.tile([P, QT, E], F32, tag="outf", name="outf")
            S['hm'] = h0_ps.tile([P, E], F32, tag="h0", name="hm")
            S['mean1'] = mean_ps.tile([P, E], F32, tag="mean", name="mean1")

        def out_head(S, h):
            attn, v_all = S['attn'], S['v']
            attnT = attnT_pool.tile([P, QT, T], F16, tag="attnT")
            atp = mm_ps.tile([P, QT, T], F32, tag="mm")
            diag = uscr_pool.tile([P, QT, P], F16, tag="diag", name="diag")
            for qt in range(QT):
                nc.scalar.activation(diag[:, qt, :], ident16[:], AF.Copy,
                                     bias=0.0, scale=S['dall'][:, qt, h:h + 1])
            for ki in range(QT):
                for qt in range(QT):
                    nc.tensor.matmul(atp[:, ki, ts(qt, P)],
                                     attn[:, qt, h, ts(ki, P)],
                                     diag[:, qt, :], start=True, stop=True)
            nc.scalar.activation(attnT[:], atp[:], AF.Copy,
                                 bias=0.0, scale=1.0)
            for ki in range(QT):
                nc.tensor.matmul(S['hm'][:], attnT[:, ki, 0:P],
                                 v_all[:, h, ki, :],
                                 start=(h == 0 and ki == 0),
                                 stop=(h == NH - 1 and ki == QT - 1))
            for ki in range(QT):
                nc.tensor.matmul(S['mean1'][:], attnT[:, ki, ts(1, P)],
                                 v_all[:, h, ki, :],
                                 start=(h == 0 and ki == 0),
                                 stop=(h == NH - 1 and ki == QT - 1))

        def finish(b, S):
            nc.vector.tensor_copy(S['outf'][0:NA, 0, :], S['hm'][0:NA, :])
            nc.scalar.activation(S['outf'][ds(NA, NA), 0, :],
                                 S['hm'][ds(NA, NA), :],
                                 AF.Copy, bias=0.0, scale=1.0)
            nc.scalar.activation(S['outf'][:, 1, :], S['mean1'][:],
                                 AF.Copy, bias=0.0, scale=1.0)
            nc.sync.dma_start(out_d[b].rearrange("(i p) e -> p i e", p=P),
                              S['outf'][:])

        # ---- 3-deep skewed pipeline -------------------------------------
        # step s: qkv+scores(s) | sparsemax(s-1) | output(s-2)
        st = [None] * BPC
        for s in range(BPC + 2):
            F = s < BPC                       # front batch exists
            M = 1 <= s <= BPC                 # mid (sparse) batch exists
            Bk = s >= 2                       # back (output) batch exists
            if F:
                st[s] = prep(s)
                alloc_qkv(st[s])
                alloc_sparse(st[s])
            if M:
                sparse_init(st[s - 1])
            if Bk:
                normalize(st[s - 2])
            for h in range(NH):
                if F:
                    qkv_head(st[s], h)
                    if h % 2 == 1:
                        scores_chunk(st[s], 0, h // 2)
                        scores_chunk(st[s], 1, h // 2)
                if M:
                    if s == BPC:
                        # no front work: compress sparse into slots 0..3 so
                        # the final batch's attn is ready before the drain
                        if h < 4:
                            sparse_piece(st[s - 1], 2 * h)
                            sparse_piece(st[s - 1], 2 * h + 1)
                    else:
                        sparse_piece(st[s - 1], h)
                if Bk:
                    out_head(st[s - 2], h)
            if Bk:
                finish(s - 2, st[s - 2])
                st[s - 2] = None
    nc.compile()
    return nc


_NC_CACHE = None


def _get_nc():
    global _NC_CACHE
    if _NC_CACHE is None:
        _NC_CACHE = build_nc()
    return _NC_CACHE


def make_in_maps(x, mask, w_q, w_k, w_v, fc_select_w, fc_select_b):
    mask_u8 = np.ascontiguousarray(mask).view(np.uint8)
    in_maps = []
    for c in range(NCORES):
        sl = slice(c * BPC, (c + 1) * BPC)
        in_maps.append({
            "x": np.ascontiguousarray(x[sl], dtype=np.float32),
            "mask": np.ascontiguousarray(mask_u8[sl]),
            "w_q": np.ascontiguousarray(w_q, dtype=np.float32),
            "w_k": np.ascontiguousarray(w_k, dtype=np.float32),
            "w_v": np.ascontiguousarray(w_v, dtype=np.float32),
            "fc_select_w": np.ascontiguousarray(fc_select_w, dtype=np.float32),
            "fc_select_b": np.ascontiguousarray(
                fc_select_b, dtype=np.float32).reshape(1, NH),
        })
    return in_maps


def kernel(x, h, mask, w_q, w_k, w_v, fc_select_w, fc_select_b, **kwargs):
    from concourse import bass_utils
    nc = _get_nc()
    in_maps = make_in_maps(x, mask, w_q, w_k, w_v, fc_select_w, fc_select_b)
    res = bass_utils.run_bass_kernel_spmd(nc, in_maps,
                                          core_ids=list(range(NCORES)))
    out = np.concatenate([res.results[c]["out"] for c in range(NCORES)], axis=0)
    return out.astype(np.float32)
